# revision 4
# baseline (speedup 1.0000x reference)
"""Trainium2 Bass kernel for nn_CrossPath (sparse_attention).

Strategy (hardcoded for B=32, N=1024, C=256, H=8, d=32, CLS=6):
  - Data-parallel over batch: 8 NeuronCores x 4 batches each, params replicated.
  - All heavy matmuls run on the PE at 1 cycle/row: bf16 for proj/attention
    paths, float32r (tf32-like) for the precision-sensitive end_proj1.
  - The linear-attention context is computed via the Gram-matrix identity
        ctx = Wk^T (act^T act) Wv * scale
    which cuts both PE work and PSUM-evacuation volume vs materializing k/v.
  - Per-head softmax over d is done on [128,128] feature blocks with an
    additive block-diagonal mask; off-diagonal (cross-head) entries go to
    exp(-100)=0, so the transposed context is exactly block-diagonal and
    attends can use full 128-wide contractions.
  - Host-side glue: shard/transpose/cast inputs, fold constant scales
    ((z1+v1)/2 -> 0.5 into ctx1 norm and u1 relu), unshard outputs.
"""

import sys

sys.path.insert(0, "/opt/trn_rl_repo")

import numpy as np
import ml_dtypes
from contextlib import ExitStack

import concourse.bass as bass
import concourse.bacc as bacc
import concourse.tile as tile
import concourse.mybir as mybir
from concourse.bass_utils import run_bass_kernel_spmd

F32 = mybir.dt.float32
F32R = mybir.dt.float32r
BF16 = mybir.dt.bfloat16
AF = mybir.ActivationFunctionType
OP = mybir.AluOpType
BF = ml_dtypes.bfloat16

N_CORES = 8
B = 32
BPC = B // N_CORES  # batches per core
DIM = 256
N = 1024
H = 8
CLS = 6
SCALE = float((DIM // H) ** -0.5)
EPS = 1e-5
NT = N // 128  # 8 token tiles
MASKV = -100.0

_CACHE = {}


def _build(bpc):
    nc = bacc.Bacc(None, target_bir_lowering=False)

    # ---- DRAM I/O (per core) ----
    d = {}
    for nm in ("x1f", "x2f", "sgf"):
        d[nm] = nc.dram_tensor(nm, [bpc, 2, 128, N], BF16, kind="ExternalInput")
    for nm in ("x1n", "x2n"):
        d[nm] = nc.dram_tensor(nm, [bpc, N, DIM], F32, kind="ExternalInput")
    for nm in ("wp1", "wp2", "wp3", "wkv1", "wkv2", "wkv3"):
        d[nm] = nc.dram_tensor(nm, [2, 128, 2 * DIM], BF16, kind="ExternalInput")
    d["we1"] = nc.dram_tensor("we1", [N, CLS], F32, kind="ExternalInput")
    d["we2"] = nc.dram_tensor("we2", [4, 128, DIM], BF16, kind="ExternalInput")
    d["tbias"] = nc.dram_tensor("tbias", [3, 512], BF16, kind="ExternalInput")
    d["fmb"] = nc.dram_tensor("fmb", [128, 3, 2], F32, kind="ExternalInput")
    d["be2row"] = nc.dram_tensor("be2row", [512], BF16, kind="ExternalInput")
    d["be1"] = nc.dram_tensor("be1", [CLS], F32, kind="ExternalInput")
    d["ones128"] = nc.dram_tensor("ones128", [128], BF16, kind="ExternalInput")
    d["ones256"] = nc.dram_tensor("ones256", [DIM], F32, kind="ExternalInput")
    d["g1"] = nc.dram_tensor("g1", [CLS], F32, kind="ExternalInput")
    d["b1"] = nc.dram_tensor("b1", [CLS], F32, kind="ExternalInput")
    d["g2"] = nc.dram_tensor("g2", [DIM], F32, kind="ExternalInput")
    d["b2"] = nc.dram_tensor("b2", [DIM], F32, kind="ExternalInput")
    d["mask"] = nc.dram_tensor("mask", [128, 128], F32, kind="ExternalInput")
    d["idbf"] = nc.dram_tensor("idbf", [128, 128], BF16, kind="ExternalInput")
    d["idf32"] = nc.dram_tensor("idf32", [128, 128], F32, kind="ExternalInput")
    d["out1"] = nc.dram_tensor("out1", [bpc, 2, 128, CLS], F32, kind="ExternalOutput")
    d["out2"] = nc.dram_tensor("out2", [bpc, N, DIM], F32, kind="ExternalOutput")

    with tile.TileContext(nc) as tc, ExitStack() as ctx:
        _emit(nc, tc, ctx, d, bpc)
    nc.compile()
    return nc


def _bcast_ap(handle, n):
    ap = handle[:]
    return bass.AP(tensor=ap.tensor, offset=0, ap=[[0, 128], [1, n]])


def _emit(nc, tc, ctx, d, bpc):
    const = ctx.enter_context(tc.tile_pool(name="const", bufs=1))
    xin = ctx.enter_context(tc.tile_pool(name="xin", bufs=2))
    acts = ctx.enter_context(tc.tile_pool(name="acts", bufs=1))
    mid = ctx.enter_context(tc.tile_pool(name="mid", bufs=2))
    ypool = ctx.enter_context(tc.tile_pool(name="ypool", bufs=1))
    opool = ctx.enter_context(tc.tile_pool(name="opool", bufs=2))
    psm = ctx.enter_context(tc.tile_pool(name="psm", bufs=4, space="PSUM"))
    psb = ctx.enter_context(tc.tile_pool(name="psb", bufs=2, space="PSUM"))

    # ---- constants (loaded once) ----
    wp_t, wkv_t = [], []
    for nm in ("wp1", "wp2", "wp3"):
        t = const.tile([128, 2, 2 * DIM], BF16, tag=nm)
        nc.sync.dma_start(t[:], d[nm][:].rearrange("c p f -> p c f"))
        wp_t.append(t)
    for nm in ("wkv1", "wkv2", "wkv3"):
        t = const.tile([128, 2, 2 * DIM], BF16, tag=nm)
        nc.sync.dma_start(t[:], d[nm][:].rearrange("c p f -> p c f"))
        wkv_t.append(t)
    we1_t = const.tile([128, NT, CLS], F32R, tag="we1")
    nc.sync.dma_start(we1_t[:], d["we1"][:].rearrange("(t p) c -> p t c", p=128).bitcast(F32R))
    we2_t = const.tile([128, 4, DIM], BF16, tag="we2")
    nc.sync.dma_start(we2_t[:], d["we2"][:].rearrange("c p f -> p c f"))
    tbias_t = const.tile([1, 3, 512], BF16, tag="tbias")
    nc.sync.dma_start(tbias_t[:], d["tbias"][:].unsqueeze(0))
    fmb_t = const.tile([128, 3, 2], F32, tag="fmb")
    nc.sync.dma_start(fmb_t[:], d["fmb"][:])
    be2row_t = const.tile([1, 512], BF16, tag="be2row")
    nc.sync.dma_start(be2row_t[:], d["be2row"][:].unsqueeze(0))
    be1_t = const.tile([1, CLS], F32R, tag="be1")
    nc.sync.dma_start(be1_t[:], d["be1"][:].unsqueeze(0).bitcast(F32R))
    ones128_t = const.tile([1, 128], BF16, tag="ones128")
    nc.sync.dma_start(ones128_t[:], d["ones128"][:].unsqueeze(0))
    ones256_t = const.tile([1, DIM], F32R, tag="ones256")
    nc.sync.dma_start(ones256_t[:], d["ones256"][:].unsqueeze(0).bitcast(F32R))
    mask_t = const.tile([128, 128], F32, tag="mask")
    nc.sync.dma_start(mask_t[:], d["mask"][:])
    idbf_t = const.tile([128, 128], BF16, tag="idbf")
    nc.sync.dma_start(idbf_t[:], d["idbf"][:])
    idf32_t = const.tile([128, 128], F32, tag="idf32")
    nc.sync.dma_start(idf32_t[:], d["idf32"][:])
    g1_t = const.tile([128, CLS], F32, tag="g1")
    nc.gpsimd.dma_start(g1_t[:], _bcast_ap(d["g1"], CLS))
    b1_t = const.tile([128, CLS], F32, tag="b1")
    nc.gpsimd.dma_start(b1_t[:], _bcast_ap(d["b1"], CLS))
    g2_t = const.tile([128, DIM], F32, tag="g2")
    nc.gpsimd.dma_start(g2_t[:], _bcast_ap(d["g2"], DIM))
    b2_t = const.tile([128, DIM], F32, tag="b2")
    nc.gpsimd.dma_start(b2_t[:], _bcast_ap(d["b2"], DIM))
    eps_t = const.tile([128, 1], F32, tag="eps")
    nc.vector.memset(eps_t[:], EPS)

    for b in range(bpc):
        # ---- load inputs ----
        xf = []
        for i, nm in enumerate(("x1f", "x2f", "sgf")):
            t = xin.tile([128, 2, N], BF16, tag=nm)
            nc.sync.dma_start(t[:], d[nm][b].rearrange("c p n -> p c n"))
            xf.append(t)
        x1n = xin.tile([128, NT, DIM], F32, tag="x1n")
        nc.sync.dma_start(x1n[:], d["x1n"][b].rearrange("(t p) c -> p t c", p=128))
        x2n = xin.tile([128, NT, DIM], F32, tag="x2n")
        nc.sync.dma_start(x2n[:], d["x2n"][b].rearrange("(t p) c -> p t c", p=128))

        # ---- projections ----
        # token-major halves (y1, y2, u3) -> atok[i]; feature-major (u1, u2, y3) -> qfm[i]
        atok, qfm = [], []
        for i in range(3):
            tok_lo = 0 if i < 2 else 2 * DIM - DIM  # y half for 0,1 ; u half for 2
            at = acts.tile([128, NT, DIM], BF16, tag=f"atok{i}")
            for pair in range(NT // 2):
                ps = psm.tile([128, 512], F32, tag="m")
                for sub in range(2):
                    nt = 2 * pair + sub
                    dst = ps[:, sub * DIM : (sub + 1) * DIM]
                    for ch in range(2):
                        nc.tensor.matmul(
                            dst,
                            xf[i][:, ch, nt * 128 : (nt + 1) * 128],
                            wp_t[i][:, ch, tok_lo : tok_lo + DIM],
                            start=(ch == 0),
                            stop=False,
                        )
                    nc.tensor.matmul(
                        dst, ones128_t[:], tbias_t[:, i, 0:DIM], start=False, stop=True
                    )
                nc.scalar.activation(
                    at[:, 2 * pair : 2 * pair + 2, :].rearrange("p a b -> p (a b)"),
                    ps[:],
                    AF.Relu,
                )
            atok.append(at)

            fm_lo = 2 * DIM - DIM if i < 2 else 0
            sc = 0.5 if i == 0 else 1.0
            qt = acts.tile([128, 2, N], BF16, tag=f"qfm{i}")
            for cc in range(2):
                ps = psb.tile([128, N], F32, tag="big")
                for nh in range(2):
                    dst = ps[:, nh * 512 : (nh + 1) * 512]
                    for ch in range(2):
                        nc.tensor.matmul(
                            dst,
                            wp_t[i][:, ch, fm_lo + cc * 128 : fm_lo + (cc + 1) * 128],
                            xf[i][:, ch, nh * 512 : (nh + 1) * 512],
                            start=(ch == 0),
                            stop=(ch == 1),
                        )
                nc.scalar.activation(
                    qt[:, cc, :], ps[:], AF.Relu, bias=fmb_t[:, i, cc : cc + 1], scale=sc
                )
            qfm.append(qt)

        # ---- Gram -> T2 -> ctxT -> softmax -> ctx ----
        ctxs = []
        for i in range(3):
            gps = psm.tile([128, 512], F32, tag="m")
            for s in range(2):
                dst = gps[:, s * DIM : (s + 1) * DIM]
                for nt in range(NT):
                    nc.tensor.matmul(
                        dst,
                        atok[i][:, nt, s * 128 : (s + 1) * 128],
                        atok[i][:, nt, :],
                        start=(nt == 0),
                        stop=(nt == NT - 1),
                    )
            gsb = mid.tile([128, 2, DIM], BF16, tag="gsb")
            nc.scalar.activation(gsb[:].rearrange("p a b -> p (a b)"), gps[:], AF.Copy)

            t2ps = psm.tile([128, 512], F32, tag="m")
            for s in range(2):
                dst = t2ps[:, s * DIM : (s + 1) * DIM]
                for ch in range(2):
                    nc.tensor.matmul(
                        dst,
                        gsb[:, ch, s * 128 : (s + 1) * 128],
                        wkv_t[i][:, ch, 0:DIM],
                        start=(ch == 0),
                        stop=(ch == 1),
                    )
            t2sb = mid.tile([128, 2, DIM], BF16, tag="t2sb")
            nc.scalar.activation(t2sb[:].rearrange("p a b -> p (a b)"), t2ps[:], AF.Copy)

            cxsb = mid.tile([128, 2, 128], BF16, tag="cxsb")
            for h in range(2):
                cps = psm.tile([128, 128], F32, tag="m")
                for ch in range(2):
                    nc.tensor.matmul(
                        cps[:],
                        wkv_t[i][:, ch, DIM + h * 128 : DIM + (h + 1) * 128],
                        t2sb[:, ch, h * 128 : (h + 1) * 128],
                        start=(ch == 0),
                        stop=(ch == 1),
                    )
                # softmax over d (free dim), per 32-block via additive mask
                tmp = mid.tile([128, 128], F32, tag="smx")
                nc.vector.scalar_tensor_tensor(
                    out=tmp[:], in0=cps[:], scalar=SCALE, in1=mask_t[:],
                    op0=OP.mult, op1=OP.add,
                )
                pex = mid.tile([128, 128], F32, tag="pex")
                ssum = mid.tile([128, 1], F32, tag="ssum")
                nc.scalar.activation(pex[:], tmp[:], AF.Exp, accum_out=ssum[:])
                rs = mid.tile([128, 1], F32, tag="rs")
                nc.vector.reciprocal(rs[:], ssum[:])
                cxT = mid.tile([128, 128], BF16, tag="cxT")
                if i == 0:
                    nc.vector.tensor_scalar(
                        out=cxT[:], in0=pex[:], scalar1=rs[:], scalar2=0.5,
                        op0=OP.mult, op1=OP.mult,
                    )
                else:
                    nc.vector.tensor_scalar_mul(cxT[:], pex[:], rs[:])
                tps = psm.tile([128, 128], BF16, tag="m")
                nc.tensor.transpose(tps[:], cxT[:], idbf_t[:])
                nc.vector.tensor_copy(cxsb[:, h, :], tps[:])
            ctxs.append(cxsb)

        # ---- attends ----
        # v1 + z1 (token-major, accumulated) -> y1o = . + x1
        y1o = ypool.tile([128, NT, DIM], F32R, tag="y1o")
        for pair in range(NT // 2):
            ps = psm.tile([128, 512], F32, tag="m")
            for sub in range(2):
                nt = 2 * pair + sub
                for h in range(2):
                    dst = ps[:, sub * DIM + h * 128 : sub * DIM + (h + 1) * 128]
                    nc.tensor.matmul(
                        dst, qfm[0][:, h, nt * 128 : (nt + 1) * 128], ctxs[2][:, h, :],
                        start=True, stop=False,
                    )
                    nc.tensor.matmul(
                        dst, qfm[2][:, h, nt * 128 : (nt + 1) * 128], ctxs[0][:, h, :],
                        start=False, stop=True,
                    )
            nc.vector.tensor_add(
                y1o[:, 2 * pair : 2 * pair + 2, :].rearrange("p a b -> p (a b)"),
                ps[:],
                x1n[:, 2 * pair : 2 * pair + 2, :].rearrange("p a b -> p (a b)"),
            )

        # z2 (q=y3, ctx2) -> y2o chunks 0,1 ; v2 (q=u2, ctx3) -> chunks 2,3
        y2o = ypool.tile([128, 4, N], BF16, tag="y2o")
        for (qi, ci, base) in ((2, 1, 0), (1, 2, 2)):
            for h in range(2):
                ps = psb.tile([128, N], F32, tag="big")
                for nh in range(2):
                    nc.tensor.matmul(
                        ps[:, nh * 512 : (nh + 1) * 512],
                        ctxs[ci][:, h, :],
                        qfm[qi][:, h, nh * 512 : (nh + 1) * 512],
                        start=True, stop=True,
                    )
                nc.scalar.activation(y2o[:, base + h, :], ps[:], AF.Copy)

        # ---- end_proj1: t^T = We1^T @ y1o + be1, transpose, LN over CLS ----
        tps = psm.tile([CLS, DIM], F32, tag="m")
        for nt in range(NT):
            nc.tensor.matmul(
                tps[:], we1_t[:, nt, :], y1o[:, nt, :], start=(nt == 0), stop=False
            )
        nc.tensor.matmul(tps[:], be1_t[:], ones256_t[:], start=False, stop=True)
        tsb = mid.tile([CLS, DIM], F32, tag="tsb")
        nc.scalar.activation(tsb[:], tps[:], AF.Copy)
        mv1 = mid.tile([128, 2, 2], F32, tag="mv1")
        trp = []
        for half in range(2):
            tp = psm.tile([128, CLS], F32, tag="m")
            nc.tensor.transpose(tp[:], tsb[:, half * 128 : (half + 1) * 128], idf32_t[:CLS, :CLS])
            trp.append(tp)
            st6 = mid.tile([128, 6], F32, tag="st6")
            nc.vector.bn_stats(st6[:], tp[:])
            nc.vector.bn_aggr(mv1[:, half, :], st6[:])
        lnv1 = mid.tile([128, 2], F32, tag="lnv1")
        nc.scalar.activation(lnv1[:], mv1[:, :, 1], AF.Ln, bias=eps_t[:])
        rstd1 = mid.tile([128, 2], F32, tag="rstd1")
        nc.scalar.activation(rstd1[:], lnv1[:], AF.Exp, scale=-0.5)
        for half in range(2):
            u = mid.tile([128, CLS], F32, tag="u6")
            nc.vector.scalar_tensor_tensor(
                out=u[:], in0=trp[half][:], scalar=mv1[:, half, 0:1], in1=g1_t[:],
                op0=OP.subtract, op1=OP.mult,
            )
            o1 = opool.tile([128, CLS], F32, tag="o1")
            nc.vector.scalar_tensor_tensor(
                out=o1[:], in0=u[:], scalar=rstd1[:, half : half + 1], in1=b1_t[:],
                op0=OP.mult, op1=OP.add,
            )
            nc.sync.dma_start(d["out1"][b, half], o1[:])

        # ---- end_proj2: p2 = y2o @ We2 + be2 ; out2 = LN(x2 + p2) ----
        mv2 = mid.tile([128, NT, 2], F32, tag="mv2")
        vsb = ypool.tile([128, NT, DIM], F32, tag="vsb")
        for pair in range(NT // 2):
            ps = psm.tile([128, 512], F32, tag="m")
            for sub in range(2):
                nt = 2 * pair + sub
                dst = ps[:, sub * DIM : (sub + 1) * DIM]
                for ck in range(4):
                    nc.tensor.matmul(
                        dst,
                        y2o[:, ck, nt * 128 : (nt + 1) * 128],
                        we2_t[:, ck, :],
                        start=(ck == 0),
                        stop=False,
                    )
                nc.tensor.matmul(
                    dst, ones128_t[:], be2row_t[:, 0:DIM], start=False, stop=True
                )
            vs = vsb[:, 2 * pair : 2 * pair + 2, :].rearrange("p a b -> p (a b)")
            nc.vector.scalar_tensor_tensor(
                out=vs, in0=ps[:], scalar=0.0, in1=x2n[:, 2 * pair : 2 * pair + 2, :].rearrange("p a b -> p (a b)"),
                op0=OP.bypass, op1=OP.add,
            )
            for sub in range(2):
                nt = 2 * pair + sub
                st = mid.tile([128, 6], F32, tag="st2")
                nc.vector.bn_stats(st[:], vsb[:, nt, :])
                nc.vector.bn_aggr(mv2[:, nt, :], st[:])
        lnv2 = mid.tile([128, NT], F32, tag="lnv2")
        nc.scalar.activation(lnv2[:], mv2[:, :, 1], AF.Ln, bias=eps_t[:])
        rstd2 = mid.tile([128, NT], F32, tag="rstd2")
        nc.scalar.activation(rstd2[:], lnv2[:], AF.Exp, scale=-0.5)
        o2 = opool.tile([128, NT, DIM], F32, tag="o2")
        for nt in range(NT):
            u = mid.tile([128, DIM], F32, tag="u2")
            nc.vector.scalar_tensor_tensor(
                out=u[:], in0=vsb[:, nt, :], scalar=mv2[:, nt, 0:1], in1=g2_t[:],
                op0=OP.subtract, op1=OP.mult,
            )
            nc.vector.scalar_tensor_tensor(
                out=o2[:, nt, :], in0=u[:], scalar=rstd2[:, nt : nt + 1], in1=b2_t[:],
                op0=OP.mult, op1=OP.add,
            )
        nc.sync.dma_start(d["out2"][b].rearrange("(t p) c -> p t c", p=128), o2[:])


def _prep_params(inp):
    """Host-side param prep shared by all cores."""
    f = lambda a: np.ascontiguousarray(a, dtype=np.float32)
    bf = lambda a: np.ascontiguousarray(np.asarray(a, dtype=np.float32).astype(BF))
    p = {}
    for nm, key in (("wp1", "Wp1"), ("wp2", "Wp2"), ("wp3", "Wp3"),
                    ("wkv1", "Wkv1"), ("wkv2", "Wkv2"), ("wkv3", "Wkv3")):
        p[nm] = bf(np.asarray(inp[key]).reshape(2, 128, 2 * DIM))
    p["we1"] = f(inp["We1"])
    p["we2"] = bf(np.asarray(inp["We2"]).reshape(4, 128, DIM))
    bp1, bp2, bp3 = (np.asarray(inp[k], np.float32) for k in ("bp1", "bp2", "bp3"))
    p["tbias"] = bf(np.stack([
        np.tile(bp1[0:DIM], 2), np.tile(bp2[0:DIM], 2), np.tile(bp3[DIM:], 2)]))
    fmb = np.stack([0.5 * bp1[DIM:], bp2[DIM:], bp3[0:DIM]])  # [3, 256]
    p["fmb"] = f(fmb.reshape(3, 2, 128).transpose(2, 0, 1))  # [128, 3, 2]
    p["be2row"] = bf(np.tile(np.asarray(inp["be2"], np.float32), 2))
    p["be1"] = f(inp["be1"])
    p["ones128"] = bf(np.ones(128))
    p["ones256"] = f(np.ones(DIM))
    for nm in ("g1", "b1", "g2", "b2"):
        p[nm] = f(inp[nm])
    ii, jj = np.meshgrid(np.arange(128), np.arange(128), indexing="ij")
    p["mask"] = np.where(ii // 32 == jj // 32, 0.0, MASKV).astype(np.float32)
    p["idbf"] = bf(np.eye(128))
    p["idf32"] = f(np.eye(128))
    return p


def _run(inputs, trace=False):
    if "nc" not in _CACHE:
        _CACHE["nc"] = _build(BPC)
    nc = _CACHE["nc"]
    params = _prep_params(inputs)
    x1 = np.asarray(inputs["x1"], np.float32)
    x2 = np.asarray(inputs["x2"], np.float32)
    sg = np.asarray(inputs["segfeature"], np.float32)

    in_maps = []
    for c in range(N_CORES):
        lo, hi = c * BPC, (c + 1) * BPC
        m = dict(params)
        for nm, arr in (("x1f", x1), ("x2f", x2), ("sgf", sg)):
            m[nm] = np.ascontiguousarray(
                arr[lo:hi].transpose(0, 2, 1).reshape(BPC, 2, 128, N).astype(BF))
        m["x1n"] = np.ascontiguousarray(x1[lo:hi])
        m["x2n"] = np.ascontiguousarray(x2[lo:hi])
        in_maps.append(m)

    res = run_bass_kernel_spmd(nc, in_maps, core_ids=list(range(N_CORES)), trace=trace)
    out1 = np.concatenate([r["out1"].reshape(BPC, 2 * 128, CLS) for r in res.results])
    out2 = np.concatenate([r["out2"] for r in res.results])
    out_x1 = np.ascontiguousarray(np.swapaxes(out1, 1, 2), dtype=np.float32)
    return (out_x1, out2.astype(np.float32, copy=False)), res


def kernel(**inputs):
    outs, _ = _run(inputs, trace=False)
    return outs


# revision 8
# speedup vs baseline: 1.2446x; 1.2446x over previous
"""Trainium2 Bass kernel for nn_CrossPath (sparse_attention).

Strategy (hardcoded for B=32, N=1024, C=256, H=8, d=32, CLS=6):
  - Data-parallel over batch: 8 NeuronCores x 4 batches each, params replicated.
  - All heavy matmuls run on the PE at 1 cycle/row: bf16 for proj/attention
    paths, float32r (tf32-like) for the precision-sensitive end_proj1.
  - The linear-attention context is computed via the Gram-matrix identity
        ctx = Wk^T (act^T act) Wv * scale
    which cuts both PE work and PSUM-evacuation volume vs materializing k/v.
  - Per-head softmax over d is done on [128,128] feature blocks with an
    additive block-diagonal mask; off-diagonal (cross-head) entries go to
    exp(-100)=0, so the transposed context is exactly block-diagonal and
    attends can use full 128-wide contractions.
  - Host-side glue: shard/transpose/cast inputs, fold constant scales
    ((z1+v1)/2 -> 0.5 into ctx1 norm and u1 relu), unshard outputs.
"""

import sys

sys.path.insert(0, "/opt/trn_rl_repo")

import numpy as np
import ml_dtypes
from contextlib import ExitStack

import concourse.bass as bass
import concourse.bacc as bacc
import concourse.tile as tile
import concourse.mybir as mybir
from concourse.bass_utils import run_bass_kernel_spmd

F32 = mybir.dt.float32
F32R = mybir.dt.float32r
BF16 = mybir.dt.bfloat16
AF = mybir.ActivationFunctionType
OP = mybir.AluOpType
BF = ml_dtypes.bfloat16

N_CORES = 8
B = 32
BPC = B // N_CORES  # batches per core
DIM = 256
N = 1024
H = 8
CLS = 6
SCALE = float((DIM // H) ** -0.5)
EPS = 1e-5
NT = N // 128  # 8 token tiles
MASKV = -100.0

_CACHE = {}


def _patch_act_tables():
    """Force all ACT functions onto the natural_log_exp_and_others table
    set (contains relu/copy/exp/ln) so the kernel pays exactly one
    ACT_TABLE_LOAD instead of thrashing between exp/ln sets. Entries keep
    their positions so act_func_set_id indices stay valid."""
    import concourse.bacc as _bacc

    if getattr(_bacc, "_ant_tables_patched", False):
        return
    orig = _bacc.get_activation_tables

    def patched(arch):
        tabs = orig(arch)
        if "natural_log_exp_and_others" not in tabs:
            return tabs
        return {
            k: (v if k == "natural_log_exp_and_others" else set())
            for k, v in tabs.items()
        }

    _bacc.get_activation_tables = patched
    _bacc._ant_tables_patched = True


def _build(bpc):
    _patch_act_tables()
    nc = bacc.Bacc(None, target_bir_lowering=False)

    # ---- DRAM I/O (per core) ----
    d = {}
    for nm in ("x1f", "x2f", "sgf"):
        d[nm] = nc.dram_tensor(nm, [bpc, 2, 128, N], BF16, kind="ExternalInput")
    for nm in ("x1n", "x2n"):
        d[nm] = nc.dram_tensor(nm, [bpc, N, DIM], F32, kind="ExternalInput")
    for nm in ("wp1", "wp2", "wp3", "wkv1", "wkv2", "wkv3"):
        d[nm] = nc.dram_tensor(nm, [2, 128, 2 * DIM], BF16, kind="ExternalInput")
    d["we1"] = nc.dram_tensor("we1", [N, CLS], F32, kind="ExternalInput")
    d["we2"] = nc.dram_tensor("we2", [4, 128, DIM], BF16, kind="ExternalInput")
    d["tbias"] = nc.dram_tensor("tbias", [3, 512], BF16, kind="ExternalInput")
    d["fmb"] = nc.dram_tensor("fmb", [128, 3, 2], F32, kind="ExternalInput")
    d["be2row"] = nc.dram_tensor("be2row", [512], BF16, kind="ExternalInput")
    d["be1"] = nc.dram_tensor("be1", [CLS], F32, kind="ExternalInput")
    d["ones128"] = nc.dram_tensor("ones128", [128], BF16, kind="ExternalInput")
    d["ones256"] = nc.dram_tensor("ones256", [DIM], F32, kind="ExternalInput")
    d["g1"] = nc.dram_tensor("g1", [CLS], F32, kind="ExternalInput")
    d["b1"] = nc.dram_tensor("b1", [CLS], F32, kind="ExternalInput")
    d["g2"] = nc.dram_tensor("g2", [DIM], F32, kind="ExternalInput")
    d["b2"] = nc.dram_tensor("b2", [DIM], F32, kind="ExternalInput")
    d["mask"] = nc.dram_tensor("mask", [128, 128], F32, kind="ExternalInput")
    d["idbf"] = nc.dram_tensor("idbf", [128, 128], BF16, kind="ExternalInput")
    d["idf32"] = nc.dram_tensor("idf32", [128, 128], F32, kind="ExternalInput")
    d["out1"] = nc.dram_tensor("out1", [bpc, 2, 128, CLS], F32, kind="ExternalOutput")
    d["out2"] = nc.dram_tensor("out2", [bpc, N, DIM], F32, kind="ExternalOutput")

    with tile.TileContext(nc) as tc, ExitStack() as ctx:
        _emit(nc, tc, ctx, d, bpc)
    nc.compile()
    return nc


def _bcast_ap(handle, n):
    ap = handle[:]
    return bass.AP(tensor=ap.tensor, offset=0, ap=[[0, 128], [1, n]])


def _emit(nc, tc, ctx, d, bpc):
    const = ctx.enter_context(tc.tile_pool(name="const", bufs=1))
    xin = ctx.enter_context(tc.tile_pool(name="xin", bufs=2))
    acts = ctx.enter_context(tc.tile_pool(name="acts", bufs=1))
    mid = ctx.enter_context(tc.tile_pool(name="mid", bufs=2))
    ypool = ctx.enter_context(tc.tile_pool(name="ypool", bufs=1))
    opool = ctx.enter_context(tc.tile_pool(name="opool", bufs=2))
    psm = ctx.enter_context(tc.tile_pool(name="psm", bufs=4, space="PSUM"))
    psb = ctx.enter_context(tc.tile_pool(name="psb", bufs=2, space="PSUM"))

    # ---- constants (loaded once) ----
    wp_t, wkv_t = [], []
    for nm in ("wp1", "wp2", "wp3"):
        t = const.tile([128, 2, 2 * DIM], BF16, tag=nm)
        nc.sync.dma_start(t[:], d[nm][:].rearrange("c p f -> p c f"))
        wp_t.append(t)
    for nm in ("wkv1", "wkv2", "wkv3"):
        t = const.tile([128, 2, 2 * DIM], BF16, tag=nm)
        nc.sync.dma_start(t[:], d[nm][:].rearrange("c p f -> p c f"))
        wkv_t.append(t)
    we1_t = const.tile([128, NT, CLS], F32R, tag="we1")
    nc.sync.dma_start(we1_t[:], d["we1"][:].rearrange("(t p) c -> p t c", p=128).bitcast(F32R))
    we2_t = const.tile([128, 4, DIM], BF16, tag="we2")
    nc.sync.dma_start(we2_t[:], d["we2"][:].rearrange("c p f -> p c f"))
    tbias_t = const.tile([1, 3, 512], BF16, tag="tbias")
    nc.sync.dma_start(tbias_t[:], d["tbias"][:].unsqueeze(0))
    fmb_t = const.tile([128, 3, 2], F32, tag="fmb")
    nc.sync.dma_start(fmb_t[:], d["fmb"][:])
    be2row_t = const.tile([1, 512], BF16, tag="be2row")
    nc.sync.dma_start(be2row_t[:], d["be2row"][:].unsqueeze(0))
    be1_t = const.tile([1, CLS], F32R, tag="be1")
    nc.sync.dma_start(be1_t[:], d["be1"][:].unsqueeze(0).bitcast(F32R))
    ones128_t = const.tile([1, 128], BF16, tag="ones128")
    nc.sync.dma_start(ones128_t[:], d["ones128"][:].unsqueeze(0))
    ones256_t = const.tile([1, DIM], F32R, tag="ones256")
    nc.sync.dma_start(ones256_t[:], d["ones256"][:].unsqueeze(0).bitcast(F32R))
    mask_t = const.tile([128, 128], F32, tag="mask")
    nc.sync.dma_start(mask_t[:], d["mask"][:])
    idbf_t = const.tile([128, 128], BF16, tag="idbf")
    nc.sync.dma_start(idbf_t[:], d["idbf"][:])
    idf32_t = const.tile([128, 128], F32, tag="idf32")
    nc.sync.dma_start(idf32_t[:], d["idf32"][:])
    g1_t = const.tile([128, CLS], F32, tag="g1")
    nc.gpsimd.dma_start(g1_t[:], _bcast_ap(d["g1"], CLS))
    b1_t = const.tile([128, CLS], F32, tag="b1")
    nc.gpsimd.dma_start(b1_t[:], _bcast_ap(d["b1"], CLS))
    g2_t = const.tile([128, DIM], F32, tag="g2")
    nc.gpsimd.dma_start(g2_t[:], _bcast_ap(d["g2"], DIM))
    b2_t = const.tile([128, DIM], F32, tag="b2")
    nc.gpsimd.dma_start(b2_t[:], _bcast_ap(d["b2"], DIM))
    eps_t = const.tile([128, 1], F32, tag="eps")
    nc.vector.memset(eps_t[:], EPS)

    for b in range(bpc):
        # ---- load inputs ----
        xf = []
        for i, nm in enumerate(("x1f", "x2f", "sgf")):
            t = xin.tile([128, 2, N], BF16, tag=nm)
            nc.sync.dma_start(t[:], d[nm][b].rearrange("c p n -> p c n"))
            xf.append(t)
        x1n = xin.tile([128, NT, DIM], F32, tag="x1n")
        nc.sync.dma_start(x1n[:], d["x1n"][b].rearrange("(t p) c -> p t c", p=128))
        x2n = xin.tile([128, NT, DIM], F32, tag="x2n")
        nc.sync.dma_start(x2n[:], d["x2n"][b].rearrange("(t p) c -> p t c", p=128))

        # ---- projections ----
        # token-major halves (y1, y2, u3) -> atok[i]; feature-major (u1, u2, y3) -> qfm[i]
        atok, qfm = [], []
        for i in range(3):
            tok_lo = 0 if i < 2 else 2 * DIM - DIM  # y half for 0,1 ; u half for 2
            at = acts.tile([128, NT, DIM], BF16, tag=f"atok{i}")
            for pair in range(NT // 2):
                ps = psm.tile([128, 512], F32, tag="m")
                for sub in range(2):
                    nt = 2 * pair + sub
                    dst = ps[:, sub * DIM : (sub + 1) * DIM]
                    for ch in range(2):
                        nc.tensor.matmul(
                            dst,
                            xf[i][:, ch, nt * 128 : (nt + 1) * 128],
                            wp_t[i][:, ch, tok_lo : tok_lo + DIM],
                            start=(ch == 0),
                            stop=False,
                        )
                    nc.tensor.matmul(
                        dst, ones128_t[:], tbias_t[:, i, 0:DIM], start=False, stop=True
                    )
                nc.scalar.activation(
                    at[:, 2 * pair : 2 * pair + 2, :].rearrange("p a b -> p (a b)"),
                    ps[:],
                    AF.Relu,
                )
            atok.append(at)

            fm_lo = 2 * DIM - DIM if i < 2 else 0
            sc = 0.5 if i == 0 else 1.0
            qt = acts.tile([128, 2, N], BF16, tag=f"qfm{i}")
            for cc in range(2):
                ps = psb.tile([128, N], F32, tag="big")
                for nh in range(2):
                    dst = ps[:, nh * 512 : (nh + 1) * 512]
                    for ch in range(2):
                        nc.tensor.matmul(
                            dst,
                            wp_t[i][:, ch, fm_lo + cc * 128 : fm_lo + (cc + 1) * 128],
                            xf[i][:, ch, nh * 512 : (nh + 1) * 512],
                            start=(ch == 0),
                            stop=(ch == 1),
                        )
                nc.scalar.activation(
                    qt[:, cc, :], ps[:], AF.Relu, bias=fmb_t[:, i, cc : cc + 1], scale=sc
                )
            qfm.append(qt)

        # ---- Gram -> T2 -> ctxT -> softmax (vector; PE stays dense) ----
        cxTs = []
        for i in range(3):
            gps = psm.tile([128, 512], F32, tag="m")
            for s in range(2):
                dst = gps[:, s * DIM : (s + 1) * DIM]
                for nt in range(NT):
                    nc.tensor.matmul(
                        dst,
                        atok[i][:, nt, s * 128 : (s + 1) * 128],
                        atok[i][:, nt, :],
                        start=(nt == 0),
                        stop=(nt == NT - 1),
                    )
            gsb = mid.tile([128, 2, DIM], BF16, tag="gsb")
            nc.scalar.activation(gsb[:].rearrange("p a b -> p (a b)"), gps[:], AF.Copy)

            t2ps = psm.tile([128, 512], F32, tag="m")
            for s in range(2):
                dst = t2ps[:, s * DIM : (s + 1) * DIM]
                for ch in range(2):
                    nc.tensor.matmul(
                        dst,
                        gsb[:, ch, s * 128 : (s + 1) * 128],
                        wkv_t[i][:, ch, 0:DIM],
                        start=(ch == 0),
                        stop=(ch == 1),
                    )
            t2sb = mid.tile([128, 2, DIM], BF16, tag="t2sb")
            nc.scalar.activation(t2sb[:].rearrange("p a b -> p (a b)"), t2ps[:], AF.Copy)

            cxT = mid.tile([128, 2, 128], BF16, tag=f"cxT{i}")
            for h in range(2):
                cps = psm.tile([128, 128], F32, tag="m")
                for ch in range(2):
                    nc.tensor.matmul(
                        cps[:],
                        wkv_t[i][:, ch, DIM + h * 128 : DIM + (h + 1) * 128],
                        t2sb[:, ch, h * 128 : (h + 1) * 128],
                        start=(ch == 0),
                        stop=(ch == 1),
                    )
                # softmax over d (free dim), per 32-block via additive mask
                tmp = mid.tile([128, 128], F32, tag="smx")
                nc.vector.scalar_tensor_tensor(
                    out=tmp[:], in0=cps[:], scalar=SCALE, in1=mask_t[:],
                    op0=OP.mult, op1=OP.add,
                )
                pex = mid.tile([128, 128], F32, tag="pex")
                ssum = mid.tile([128, 1], F32, tag="ssum")
                nc.scalar.activation(pex[:], tmp[:], AF.Exp, accum_out=ssum[:])
                rs = mid.tile([128, 1], F32, tag="rs")
                nc.vector.reciprocal(rs[:], ssum[:])
                if i == 0:
                    nc.vector.tensor_scalar(
                        out=cxT[:, h, :], in0=pex[:], scalar1=rs[:], scalar2=0.5,
                        op0=OP.mult, op1=OP.mult,
                    )
                else:
                    nc.vector.tensor_scalar_mul(cxT[:, h, :], pex[:], rs[:])
            cxTs.append(cxT)

        # ---- transpose softmaxed ctxT blocks on PE, interleaved with the
        # attends that are ready, so the in-order PE never stalls long ----
        ctxs = [None, None, None]

        def ctx_transpose(i):
            cxsb = mid.tile([128, 2, 128], BF16, tag=f"cxsb{i}")
            for h in range(2):
                tps = psm.tile([128, 128], BF16, tag="m")
                nc.tensor.transpose(tps[:], cxTs[i][:, h, :], idbf_t[:])
                nc.vector.tensor_copy(cxsb[:, h, :], tps[:])
            ctxs[i] = cxsb

        # z2 (q=y3, ctx2) -> y2o chunks 0,1 ; v2 (q=u2, ctx3) -> chunks 2,3
        y2o = ypool.tile([128, 4, N], BF16, tag="y2o")

        def fm_attend(qi, ci, base):
            for h in range(2):
                ps = psb.tile([128, N], F32, tag="big")
                for nh in range(2):
                    nc.tensor.matmul(
                        ps[:, nh * 512 : (nh + 1) * 512],
                        ctxs[ci][:, h, :],
                        qfm[qi][:, h, nh * 512 : (nh + 1) * 512],
                        start=True, stop=True,
                    )
                nc.scalar.activation(y2o[:, base + h, :], ps[:], AF.Copy)

        ctx_transpose(0)
        ctx_transpose(1)
        fm_attend(2, 1, 0)   # z2 = y3 @ ctx2
        ctx_transpose(2)
        fm_attend(1, 2, 2)   # v2 = u2 @ ctx3

        # v1 + z1 (token-major, accumulated) -> y1o = . + x1
        y1o = ypool.tile([128, NT, DIM], F32R, tag="y1o")
        for pair in range(NT // 2):
            ps = psm.tile([128, 512], F32, tag="m")
            for sub in range(2):
                nt = 2 * pair + sub
                for h in range(2):
                    dst = ps[:, sub * DIM + h * 128 : sub * DIM + (h + 1) * 128]
                    nc.tensor.matmul(
                        dst, qfm[0][:, h, nt * 128 : (nt + 1) * 128], ctxs[2][:, h, :],
                        start=True, stop=False,
                    )
                    nc.tensor.matmul(
                        dst, qfm[2][:, h, nt * 128 : (nt + 1) * 128], ctxs[0][:, h, :],
                        start=False, stop=True,
                    )
            nc.vector.tensor_add(
                y1o[:, 2 * pair : 2 * pair + 2, :].rearrange("p a b -> p (a b)"),
                ps[:],
                x1n[:, 2 * pair : 2 * pair + 2, :].rearrange("p a b -> p (a b)"),
            )

        # ---- end_proj1: t^T = We1^T @ y1o + be1, transpose, LN over CLS ----
        tps = psm.tile([CLS, DIM], F32, tag="m")
        for nt in range(NT):
            nc.tensor.matmul(
                tps[:], we1_t[:, nt, :], y1o[:, nt, :], start=(nt == 0), stop=False
            )
        nc.tensor.matmul(tps[:], be1_t[:], ones256_t[:], start=False, stop=True)
        tsb = mid.tile([CLS, DIM], F32, tag="tsb")
        nc.scalar.activation(tsb[:], tps[:], AF.Copy)
        mv1 = mid.tile([128, 2, 2], F32, tag="mv1")
        trp = []
        for half in range(2):
            tp = psm.tile([128, CLS], F32, tag="m")
            nc.tensor.transpose(tp[:], tsb[:, half * 128 : (half + 1) * 128], idf32_t[:CLS, :CLS])
            trp.append(tp)
            st6 = mid.tile([128, 6], F32, tag="st6")
            nc.vector.bn_stats(st6[:], tp[:])
            nc.vector.bn_aggr(mv1[:, half, :], st6[:])
        lnv1 = mid.tile([128, 2], F32, tag="lnv1")
        nc.scalar.activation(lnv1[:], mv1[:, :, 1], AF.Ln, bias=eps_t[:])
        rstd1 = mid.tile([128, 2], F32, tag="rstd1")
        nc.scalar.activation(rstd1[:], lnv1[:], AF.Exp, scale=-0.5)
        for half in range(2):
            u = mid.tile([128, CLS], F32, tag="u6")
            nc.vector.scalar_tensor_tensor(
                out=u[:], in0=trp[half][:], scalar=mv1[:, half, 0:1], in1=g1_t[:],
                op0=OP.subtract, op1=OP.mult,
            )
            o1 = opool.tile([128, CLS], F32, tag="o1")
            nc.vector.scalar_tensor_tensor(
                out=o1[:], in0=u[:], scalar=rstd1[:, half : half + 1], in1=b1_t[:],
                op0=OP.mult, op1=OP.add,
            )
            nc.sync.dma_start(d["out1"][b, half], o1[:])

        # ---- end_proj2: p2 = y2o @ We2 + be2 ; out2 = LN(x2 + p2) ----
        mv2 = mid.tile([128, NT, 2], F32, tag="mv2")
        vsb = ypool.tile([128, NT, DIM], F32, tag="vsb")
        for pair in range(NT // 2):
            ps = psm.tile([128, 512], F32, tag="m")
            for sub in range(2):
                nt = 2 * pair + sub
                dst = ps[:, sub * DIM : (sub + 1) * DIM]
                for ck in range(4):
                    nc.tensor.matmul(
                        dst,
                        y2o[:, ck, nt * 128 : (nt + 1) * 128],
                        we2_t[:, ck, :],
                        start=(ck == 0),
                        stop=False,
                    )
                nc.tensor.matmul(
                    dst, ones128_t[:], be2row_t[:, 0:DIM], start=False, stop=True
                )
            vs = vsb[:, 2 * pair : 2 * pair + 2, :].rearrange("p a b -> p (a b)")
            nc.vector.scalar_tensor_tensor(
                out=vs, in0=ps[:], scalar=0.0, in1=x2n[:, 2 * pair : 2 * pair + 2, :].rearrange("p a b -> p (a b)"),
                op0=OP.bypass, op1=OP.add,
            )
            for sub in range(2):
                nt = 2 * pair + sub
                st = mid.tile([128, 6], F32, tag="st2")
                nc.vector.bn_stats(st[:], vsb[:, nt, :])
                nc.vector.bn_aggr(mv2[:, nt, :], st[:])
        lnv2 = mid.tile([128, NT], F32, tag="lnv2")
        nc.scalar.activation(lnv2[:], mv2[:, :, 1], AF.Ln, bias=eps_t[:])
        rstd2 = mid.tile([128, NT], F32, tag="rstd2")
        nc.scalar.activation(rstd2[:], lnv2[:], AF.Exp, scale=-0.5)
        o2 = opool.tile([128, NT, DIM], F32, tag="o2")
        for nt in range(NT):
            u = mid.tile([128, DIM], F32, tag="u2")
            nc.vector.scalar_tensor_tensor(
                out=u[:], in0=vsb[:, nt, :], scalar=mv2[:, nt, 0:1], in1=g2_t[:],
                op0=OP.subtract, op1=OP.mult,
            )
            nc.vector.scalar_tensor_tensor(
                out=o2[:, nt, :], in0=u[:], scalar=rstd2[:, nt : nt + 1], in1=b2_t[:],
                op0=OP.mult, op1=OP.add,
            )
        nc.sync.dma_start(d["out2"][b].rearrange("(t p) c -> p t c", p=128), o2[:])


def _prep_params(inp):
    """Host-side param prep shared by all cores."""
    f = lambda a: np.ascontiguousarray(a, dtype=np.float32)
    bf = lambda a: np.ascontiguousarray(np.asarray(a, dtype=np.float32).astype(BF))
    p = {}
    for nm, key in (("wp1", "Wp1"), ("wp2", "Wp2"), ("wp3", "Wp3"),
                    ("wkv1", "Wkv1"), ("wkv2", "Wkv2"), ("wkv3", "Wkv3")):
        p[nm] = bf(np.asarray(inp[key]).reshape(2, 128, 2 * DIM))
    p["we1"] = f(inp["We1"])
    p["we2"] = bf(np.asarray(inp["We2"]).reshape(4, 128, DIM))
    bp1, bp2, bp3 = (np.asarray(inp[k], np.float32) for k in ("bp1", "bp2", "bp3"))
    p["tbias"] = bf(np.stack([
        np.tile(bp1[0:DIM], 2), np.tile(bp2[0:DIM], 2), np.tile(bp3[DIM:], 2)]))
    fmb = np.stack([0.5 * bp1[DIM:], bp2[DIM:], bp3[0:DIM]])  # [3, 256]
    p["fmb"] = f(fmb.reshape(3, 2, 128).transpose(2, 0, 1))  # [128, 3, 2]
    p["be2row"] = bf(np.tile(np.asarray(inp["be2"], np.float32), 2))
    p["be1"] = f(inp["be1"])
    p["ones128"] = bf(np.ones(128))
    p["ones256"] = f(np.ones(DIM))
    for nm in ("g1", "b1", "g2", "b2"):
        p[nm] = f(inp[nm])
    ii, jj = np.meshgrid(np.arange(128), np.arange(128), indexing="ij")
    p["mask"] = np.where(ii // 32 == jj // 32, 0.0, MASKV).astype(np.float32)
    p["idbf"] = bf(np.eye(128))
    p["idf32"] = f(np.eye(128))
    return p


def _run(inputs, trace=False):
    if "nc" not in _CACHE:
        _CACHE["nc"] = _build(BPC)
    nc = _CACHE["nc"]
    params = _prep_params(inputs)
    x1 = np.asarray(inputs["x1"], np.float32)
    x2 = np.asarray(inputs["x2"], np.float32)
    sg = np.asarray(inputs["segfeature"], np.float32)

    in_maps = []
    for c in range(N_CORES):
        lo, hi = c * BPC, (c + 1) * BPC
        m = dict(params)
        for nm, arr in (("x1f", x1), ("x2f", x2), ("sgf", sg)):
            m[nm] = np.ascontiguousarray(
                arr[lo:hi].transpose(0, 2, 1).reshape(BPC, 2, 128, N).astype(BF))
        m["x1n"] = np.ascontiguousarray(x1[lo:hi])
        m["x2n"] = np.ascontiguousarray(x2[lo:hi])
        in_maps.append(m)

    res = run_bass_kernel_spmd(nc, in_maps, core_ids=list(range(N_CORES)), trace=trace)
    out1 = np.concatenate([r["out1"].reshape(BPC, 2 * 128, CLS) for r in res.results])
    out2 = np.concatenate([r["out2"] for r in res.results])
    out_x1 = np.ascontiguousarray(np.swapaxes(out1, 1, 2), dtype=np.float32)
    return (out_x1, out2.astype(np.float32, copy=False)), res


def kernel(**inputs):
    outs, _ = _run(inputs, trace=False)
    return outs
